# revision 8
# baseline (speedup 1.0000x reference)
"""Trainium2 Bass kernel for CustomYOLOLoss (N=512, S=52, NB=3), 8-core data parallel.

v2: fp16 compute pipeline engineered around DVE perf modes:
  - tensor_tensor on packed fp16 runs 2x (0.52 ns/el/partition);
    tensor_scalar/copy run 4x; copy_predicated/reduce/custom run 1x.
  - ACT engine does sigmoid (12 planes), softplus(+/-c) (6 planes) and all
    column accumulations (Copy + accum_out).
  - GPSIMD computes the target-derived planes (corners, areas, obj mask)
    and the bce0-sum plane A.
  - DVE does geometry + selection with NO copy_predicated: the responsible
    box is resolved into a one-hot (obj-masked) weight ok_b built from
    strict-greater masks, and selected values come from 3-term dots.

Math per cell (nb=3): pred box b: p = sigmoid(x), corners a1/a2; target
corners b1/b2 (gpsimd).  iw = min(a2,b2)-max(a1,b1); inter = relu(iwx)relu(iwy);
V = aa + areab + eps;  U = V - inter (= union+eps);  iou = inter * recip(U).
argmax first-wins: m1 = iou1>iou0, m2 = iou2>max(iou0,iou1).
ok = obj * onehot(argmax).  Enclosure via D = pw - iw:  enc_x*obj =
obj*tw + sum_b ok_b Dx_b, enc = encx*ency,  R-term = (sum ok U) * recip(enc+E0).
giou accounting: sum obj*giou = IOUACC - NOBJ + G2ACC.

Accumulator columns per chunk (ACT accum, fp32):
  0: S    = sum b0 (all cells, all boxes)        b0 = softplus(c) = bce(c,0)
  1: T    = sum obj * A            A = sum_b b0_b
  2: UB   = sum ok . b0            (= sum obj * b0_sel)
  3: NO   = sum ok . bp            bp = softplus(-c) = bce(c,1)
  4: IOU  = sum ok . iou
  5: G2   = sum (ok.U) * recip(enc+E0)
  6: NOBJ = sum obj
Host: num1 = S-T, num2 = T-UB, loss_obj = NO/n, num_bbox = 2n - IOU - G2.
"""

import os
import numpy as np

import concourse.bass as bass
import concourse.bacc as bacc
import concourse.mybir as mybir
import concourse.tile as tile
from concourse.bass_utils import run_bass_kernel_spmd

F32 = mybir.dt.float32
F16 = mybir.dt.float16
AF = mybir.ActivationFunctionType
ALU = mybir.AluOpType

N, S, NB = 512, 52, 3
CORES = 8
NPC = N // CORES
P = 128
CELLS = NPC * S * S                   # 173056
X = CELLS // P                        # 1352
EPS = 1e-7
E0 = 1e-4                             # enc epsilon (fp16-safe, vs ref 1e-7)

F = int(os.environ.get("YOLO_F", "676"))
REPEAT = int(os.environ.get("YOLO_REPEAT", "1"))
NCH = X // F
NACC = 8                              # accum columns per chunk (7 used)

_nc_cache = {}


def build_nc():
    key = (F, REPEAT)
    if key in _nc_cache:
        return _nc_cache[key]
    nc = bacc.Bacc(trn_type="TRN2", target_bir_lowering=False)
    inp = nc.dram_tensor("input", [P, X * 15], F32, kind="ExternalInput")
    tgt = nc.dram_tensor("target", [P, X * 5], F32, kind="ExternalInput")
    out = nc.dram_tensor("out", [P, NACC * NCH], F32, kind="ExternalOutput")

    with tile.TileContext(nc) as tc:
        with (
            tc.tile_pool(name="dma", bufs=1) as dma_pool,
            tc.tile_pool(name="act1", bufs=1) as act1,     # pxy/ps single-buf
            tc.tile_pool(name="act2", bufs=2) as act2,     # b0/bp double-buf
            tc.tile_pool(name="tgt1", bufs=1) as tgt1,
            tc.tile_pool(name="tgt2", bufs=2) as tgt2,
            tc.tile_pool(name="work", bufs=1) as work,     # DVE-internal
            tc.tile_pool(name="accp", bufs=1) as accp,
        ):
            acc = accp.tile([P, NACC * NCH], F32)

            for rep in range(REPEAT):
              for ch in range(NCH):
                col0 = ch * NACC

                def acol(i, col0=col0):
                    return acc[:, col0 + i:col0 + i + 1]

                # ---- DMA input chunk
                tin = dma_pool.tile([P, F * 15], F32, tag="tin")
                ttg = dma_pool.tile([P, F * 5], F32, tag="ttg")
                nc.sync.dma_start(tin[:], inp[:, ch * F * 15:(ch + 1) * F * 15])
                nc.sync.dma_start(ttg[:], tgt[:, ch * F * 5:(ch + 1) * F * 5])

                tin_r = tin[:].rearrange("p (f b c) -> p c b f", b=3, c=5)
                ttg_r = ttg[:].rearrange("p (f c) -> p c f", c=5)

                # ---- ACT stage 1: sigmoid + softplus (fp32 in, fp16 out)
                # PXY = [px0 px1 px2 | py0 py1 py2], PS = [pw3 | ph3]
                pxy = act1.tile([P, 6 * F], F16, tag="pxy")
                ps = act1.tile([P, 6 * F], F16, tag="ps")
                b0 = act2.tile([P, 3 * F], F16, tag="b0")
                bp = act2.tile([P, 3 * F], F16, tag="bp")
                et = act1.tile([P, 3 * F], F16, tag="et")
                c16 = act2.tile([P, 3 * F], F16, tag="c16")
                pxy_v = pxy[:].rearrange("p (c b f) -> p c b f", c=2, b=3)
                ps_v = ps[:].rearrange("p (c b f) -> p c b f", c=2, b=3)
                bp_v = bp[:].rearrange("p (b f) -> p b f", b=3)
                et_v = et[:].rearrange("p (b f) -> p b f", b=3)
                c16_v = c16[:].rearrange("p (b f) -> p b f", b=3)
                nc.scalar.activation(pxy_v, tin_r[:, 1:3], AF.Sigmoid)
                nc.scalar.activation(ps_v, tin_r[:, 3:5], AF.Sigmoid)
                # bp = softplus(-c) = ln(1 + exp(-c));  b0 = softplus(c) = c + bp
                nc.scalar.activation(et_v, tin_r[:, 0], AF.Exp, scale=-1.0)
                nc.scalar.activation(bp_v, et_v, AF.Ln, bias=1.0)
                nc.gpsimd.tensor_copy(c16_v, tin_r[:, 0])
                nc.vector.tensor_tensor(b0[:], c16[:], bp[:], ALU.add)

                # ---- GPSIMD stage 1: target-derived planes (fp16 out)
                b1 = tgt1.tile([P, 2 * F], F16, tag="b1")   # [b1x | b1y]
                b2 = tgt1.tile([P, 2 * F], F16, tag="b2")
                twh = tgt2.tile([P, 2 * F], F16, tag="twh")  # [tw | th]
                abe = tgt2.tile([P, F], F16, tag="abe")      # tw*th + ~eps
                obj = tgt2.tile([P, F], F16, tag="obj")
                av = tgt1.tile([P, F], F16, tag="av")        # A = sum_b b0_b
                oav = tgt2.tile([P, F], F16, tag="oav")      # obj * A
                b1_v = b1[:].rearrange("p (c f) -> p c f", c=2)
                b2_v = b2[:].rearrange("p (c f) -> p c f", c=2)
                twh_v = twh[:].rearrange("p (c f) -> p c f", c=2)
                t22 = tgt1.tile([P, 2 * F], F16, tag="t22")  # tw/2 | th/2
                t22_v = t22[:].rearrange("p (c f) -> p c f", c=2)
                nc.gpsimd.tensor_scalar(t22_v, ttg_r[:, 3:5], 0.5, None, ALU.mult)
                nc.gpsimd.tensor_copy(twh_v, ttg_r[:, 3:5])
                nc.gpsimd.tensor_tensor(b1_v, ttg_r[:, 1:3], t22_v, ALU.subtract)
                nc.gpsimd.tensor_tensor(b2_v, ttg_r[:, 1:3], t22_v, ALU.add)
                # areab + eps
                nc.gpsimd.tensor_tensor(abe[:], ttg_r[:, 3], ttg_r[:, 4], ALU.mult)
                nc.gpsimd.tensor_scalar(abe[:], abe[:], EPS, None, ALU.add)
                nc.gpsimd.tensor_scalar(obj[:], ttg_r[:, 0], 0.0, None, ALU.is_gt)
                # A = b0_0 + b0_1 + b0_2 ; oA = obj * A
                nc.gpsimd.tensor_tensor(av[:], b0[:, 0:F], b0[:, F:2 * F], ALU.add)
                nc.gpsimd.tensor_tensor(av[:], av[:], b0[:, 2 * F:3 * F], ALU.add)
                nc.gpsimd.tensor_tensor(oav[:], obj[:], av[:], ALU.mult)

                # ---- DVE geometry (all fp16 packed -> 2x; TS -> 4x)
                pw2 = work.tile([P, 6 * F], F16, tag="pw2")
                a1 = work.tile([P, 6 * F], F16, tag="a1")
                a2 = work.tile([P, 6 * F], F16, tag="a2")
                nc.vector.tensor_scalar(pw2[:], ps[:], 0.5, None, ALU.mult)
                nc.vector.tensor_tensor(a1[:], pxy[:], pw2[:], ALU.subtract)
                nc.vector.tensor_tensor(a2[:], pxy[:], pw2[:], ALU.add)
                a1_v = a1[:].rearrange("p (a b f) -> p a b f", a=2, b=3)
                a2_v = a2[:].rearrange("p (a b f) -> p a b f", a=2, b=3)
                b1_bc = b1_v.unsqueeze(2).broadcast_to((P, 2, 3, F))
                b2_bc = b2_v.unsqueeze(2).broadcast_to((P, 2, 3, F))
                nc.vector.tensor_tensor(a1_v, a1_v, b1_bc, ALU.max)   # lt
                nc.vector.tensor_tensor(a2_v, a2_v, b2_bc, ALU.min)   # rb
                nc.vector.tensor_tensor(a1[:], a2[:], a1[:], ALU.subtract)  # iw raw
                nc.vector.tensor_scalar(pw2[:], a1[:], 0.0, None, ALU.max)  # relu
                inter = work.tile([P, 3 * F], F16, tag="inter")
                nc.vector.tensor_tensor(inter[:], pw2[:, 0:3 * F],
                                        pw2[:, 3 * F:6 * F], ALU.mult)
                aa = work.tile([P, 3 * F], F16, tag="aa")
                nc.vector.tensor_tensor(aa[:], ps[:, 0:3 * F], ps[:, 3 * F:6 * F],
                                        ALU.mult)
                abe_bc = abe[:].unsqueeze(1).broadcast_to((P, 3, F))
                aa_v = aa[:].rearrange("p (b f) -> p b f", b=3)
                nc.vector.tensor_tensor(aa_v, aa_v, abe_bc, ALU.add)  # V
                u3 = work.tile([P, 3 * F], F32, tag="u3")
                nc.vector.tensor_tensor(u3[:], aa[:], inter[:], ALU.subtract)
                # D = pw - iw (for enclosure of the selected box); reuse a2
                dxy = a2
                nc.vector.tensor_tensor(dxy[:], ps[:], a1[:], ALU.subtract)

                # ---- iou (in place over inter) + masks
                ru3 = work.tile([P, 3 * F], F32, tag="ru3")
                nc.vector.reciprocal_approx_fast(ru3[:], u3[:])
                iou = inter
                nc.vector.tensor_tensor(iou[:], inter[:], ru3[:], ALU.mult)
                mk = work.tile([P, 3 * F], F16, tag="mk")  # m1 | mx/okx | m2
                nc.vector.tensor_tensor(mk[:, 0:F], iou[:, F:2 * F], iou[:, 0:F],
                                        ALU.is_gt)
                nc.vector.tensor_tensor(mk[:, F:2 * F], iou[:, 0:F], iou[:, F:2 * F],
                                        ALU.max)
                nc.vector.tensor_tensor(mk[:, 2 * F:3 * F], iou[:, 2 * F:3 * F],
                                        mk[:, F:2 * F], ALU.is_gt)

                # ---- one-hot obj-masked weights ok_b (reuse aa; okx in mx slot)
                ok = aa
                okx = mk[:, F:2 * F]
                nc.vector.tensor_tensor(ok[:, 2 * F:3 * F], obj[:],
                                        mk[:, 2 * F:3 * F], ALU.mult)
                nc.vector.tensor_tensor(okx, obj[:], ok[:, 2 * F:3 * F],
                                        ALU.subtract)
                nc.vector.tensor_tensor(ok[:, F:2 * F], okx, mk[:, 0:F],
                                        ALU.mult)
                nc.vector.tensor_tensor(ok[:, 0:F], okx, ok[:, F:2 * F],
                                        ALU.subtract)

                # ---- sum-dots (DVE mult + ACT accum)
                sc = work.tile([P, 3 * F], F16, tag="sc")
                nc.vector.tensor_tensor(sc[:], ok[:], b0[:], ALU.mult)
                nc.scalar.activation(sc[:], sc[:], AF.Copy, accum_out=acol(2))
                sc2 = work.tile([P, 3 * F], F16, tag="sc2")
                nc.vector.tensor_tensor(sc2[:], ok[:], bp[:], ALU.mult)
                nc.scalar.activation(sc2[:], sc2[:], AF.Copy, accum_out=acol(3))
                sc3 = work.tile([P, 3 * F], F16, tag="sc3")
                nc.vector.tensor_tensor(sc3[:], ok[:], iou[:], ALU.mult)
                nc.scalar.activation(sc3[:], sc3[:], AF.Copy, accum_out=acol(4))

                # ---- per-cell dots: oU, oDx, oDy
                tu = work.tile([P, 3 * F], F16, tag="tu")
                nc.vector.tensor_tensor(tu[:], ok[:], u3[:], ALU.mult)
                pc = work.tile([P, 8 * F], F16, tag="pc")
                # pc planes: 0 oU, 1 oDx, 2 oDy, 3 encx, 4 ency, 5 enc, 6 g2s, 7 spare
                nc.vector.tensor_tensor(pc[:, 0:F], tu[:, 0:F], tu[:, F:2 * F],
                                        ALU.add)
                nc.vector.tensor_tensor(pc[:, 0:F], pc[:, 0:F], tu[:, 2 * F:3 * F],
                                        ALU.add)
                td = pw2
                okd_bc = ok[:].rearrange("p (b f) -> p b f", b=3)
                okd_bc = okd_bc.unsqueeze(1).broadcast_to((P, 2, 3, F))
                dxy_v = dxy[:].rearrange("p (a b f) -> p a b f", a=2, b=3)
                td_v = td[:].rearrange("p (a b f) -> p a b f", a=2, b=3)
                nc.vector.tensor_tensor(td_v, dxy_v, okd_bc, ALU.mult)
                nc.vector.tensor_tensor(
                    pc[:, F:3 * F].rearrange("p (a f) -> p a f", a=2),
                    td_v[:, :, 0, :], td_v[:, :, 1, :], ALU.add)
                nc.vector.tensor_tensor(
                    pc[:, F:3 * F].rearrange("p (a f) -> p a f", a=2),
                    pc[:, F:3 * F].rearrange("p (a f) -> p a f", a=2),
                    td_v[:, :, 2, :], ALU.add)

                # ---- enclosure + R-term
                otw = work.tile([P, 2 * F], F16, tag="otw")
                otw_v = otw[:].rearrange("p (c f) -> p c f", c=2)
                obj_bc2 = obj[:].unsqueeze(1).broadcast_to((P, 2, F))
                nc.vector.tensor_tensor(otw_v, obj_bc2, twh_v, ALU.mult)
                nc.vector.tensor_tensor(
                    pc[:, 3 * F:5 * F], otw[:], pc[:, F:3 * F], ALU.add)
                nc.vector.tensor_tensor(pc[:, 5 * F:6 * F], pc[:, 3 * F:4 * F],
                                        pc[:, 4 * F:5 * F], ALU.mult)
                ence = work.tile([P, F], F32, tag="ence")
                nc.vector.tensor_scalar(ence[:], pc[:, 5 * F:6 * F], E0, None,
                                        ALU.add)
                re1 = work.tile([P, F], F32, tag="re1")
                nc.vector.reciprocal_approx_fast(re1[:], ence[:])
                nc.vector.tensor_tensor(pc[:, 6 * F:7 * F], pc[:, 0:F], re1[:],
                                        ALU.mult)
                nc.scalar.activation(pc[:, 6 * F:7 * F], pc[:, 6 * F:7 * F],
                                     AF.Copy, accum_out=acol(5))

                # ---- remaining accums (ACT)
                nc.scalar.activation(b0[:], b0[:], AF.Copy, accum_out=acol(0))
                nc.scalar.activation(oav[:], oav[:], AF.Copy, accum_out=acol(1))
                nc.scalar.activation(obj[:], obj[:], AF.Copy, accum_out=acol(6))

            nc.gpsimd.dma_start(out[:], acc[:])

    nc.compile()
    _nc_cache[key] = nc
    return nc


def kernel(input, target):
    nc = build_nc()
    in_maps = []
    for c in range(CORES):
        sl = slice(c * NPC, (c + 1) * NPC)
        in_maps.append({
            "input": np.ascontiguousarray(input[sl]).reshape(P, X * 15),
            "target": np.ascontiguousarray(target[sl]).reshape(P, X * 5),
        })
    res = run_bass_kernel_spmd(nc, in_maps, core_ids=list(range(CORES)))
    total = np.zeros(NACC, dtype=np.float64)
    for r in res.results:
        total += r["out"].reshape(P, NCH, NACC).sum(axis=(0, 1), dtype=np.float64)
    S_all, T, UB, NO, IOU, G2, NOBJ = total[:7]
    n_obj = NOBJ
    n_noobj = float(N * S * S) - n_obj
    num1 = S_all - T
    num2 = T - UB
    num_bbox = 2.0 * n_obj - IOU - G2
    loss_noobj = num1 / (n_noobj * NB) + num2 / (n_obj * (NB - 1))
    loss_bbox = num_bbox / n_obj
    loss_obj = NO / n_obj
    loss = loss_obj + loss_bbox + loss_noobj
    return (np.float32(loss), np.float32(loss_noobj), np.float32(loss_bbox),
            np.float32(loss_obj))
